# revision 28
# baseline (speedup 1.0000x reference)
"""3x3 valid cross-correlation of a 4096x4096 fp32 image + scalar bias,
sharded row-wise across 8 TRN2 NeuronCores.

bf16 datapath (harness gate is rel_err < 2e-2; bf16 lands ~5e-3):
  - x is cast to bf16 on host -> load DMA traffic halves (4.2 MB/core).
  - Matmuls run bf16 x bf16 -> fp32 PSUM at 1 cycle/column.
  - Output is stored as bf16 (4.2 MB/core) and upcast to fp32 on host.

Strategy per core (512 output rows, 514 input rows incl. 2-row halo taken
host-side via overlapping slices -- no device collectives):
  - Row panels of 128 input rows -> 126 output rows (banded matmul):
    out[m, n] = sum_dc sum_dr w[dr, dc] * x[m+dr, n+dc]
    For each kernel column dc, a banded stationary matrix
    B_dc[k, m] = w[k-m, dc] (k-m in 0..2) gives
    psum[m, n] += sum_k B_dc[k, m] * x[k, n+dc].
  - The 8-row tail (rows 504..512) is packed 12 column-blocks deep into
    the PE contraction dim: stationary [120, 96] block-diagonal banded
    matrix, moving operand [120, 345] gathered on host so partition
    10j+i = x[504+i, 341j:341j+345]. 3 matmuls of 343 columns replace a
    full 3x4094-column pass (12x fewer PE cycles for the tail).
  - Scheduling is dominated by the NC activity manager (HAM): DMA is
    capped at ~230-270 GB/s and the PE at ~1.2 GHz until ~5-6us of
    sustained activity earn the full-rate grant (~430 GB/s, 2.4 GHz),
    and any PE idle gap early in the run triggers a half-rate (k=4/8)
    throttle spiral. The warmup matmuls therefore bridge the PE from
    its first possible cycle (~8us, after the fixed SPMD prologue)
    until panel 0's load semaphore fires (~13-14.5us), so the matmul
    stream runs gap-free at full clock (215ns per 512-col matmul).
  - Bias is fused into the PSUM->SBUF drain (ScalarE activation bias for
    even chunks, VectorE tensor_scalar_add for odd ones, both converting
    to bf16).
  - Store rows are padded to 4096 cols (single 8KB packet per row) and
    panels alternate between the gpsimd and sync queues: one queue
    sustains only ~8 concurrent write streams (~200 GB/s).
  - Last core overlaps core 6 by 2 rows so all cores run an identical
    514-row program (4094 = 8*512 - 2).
"""

import numpy as np
import ml_dtypes

import concourse.bacc as bacc
import concourse.mybir as mybir
from concourse import tile
from concourse.bass_utils import run_bass_kernel_spmd

H, W = 4096, 4096
KH, KW = 3, 3
OH, OW = H - KH + 1, W - KW + 1  # 4094, 4094
NCORES = 8
ROWS_PER_CORE = 512              # output rows computed per core
IN_ROWS = ROWS_PER_CORE + KH - 1  # 514 input rows per core
PANEL_OUT = 126                  # output rows per full 128-input-row panel
N_FULL_PANELS = 4                # 4 * 126 = 504
TAIL_OUT = ROWS_PER_CORE - N_FULL_PANELS * PANEL_OUT  # 8
TAIL_IN = TAIL_OUT + KH - 1      # 10
COLS_PER_MM = 512                # PSUM-bank max (512 fp32)
CHUNK = 1024                     # PSUM chunk = 2 banks
# Packed tail geometry: 12 column blocks, stride 341, 343 output columns
# each; 341*11 + 343 = 4094 exactly, and input reads stop at 4096.
TJ = 12
TSTRIDE = 341
TN = 343
WARMUP_MM = 13

_F32 = mybir.dt.float32
_BF16 = mybir.dt.bfloat16
BF = ml_dtypes.bfloat16

_PROGRAM_CACHE = None
last_results = None  # BassKernelResults of the most recent kernel() call


def _build_program():
    nc = bacc.Bacc(
        "TRN2", target_bir_lowering=False, debug=False, num_devices=NCORES
    )
    x = nc.dram_tensor("x", [IN_ROWS, W], _BF16, kind="ExternalInput")
    xt_p = nc.dram_tensor("xt", [TJ * TAIL_IN, TN + KW - 1], _BF16,
                          kind="ExternalInput")
    w = nc.dram_tensor("w", [128, KW * PANEL_OUT], _BF16, kind="ExternalInput")
    wt_p = nc.dram_tensor("wt", [TJ * TAIL_IN, KW * TJ * TAIL_OUT], _BF16,
                          kind="ExternalInput")
    b = nc.dram_tensor("b", [128, 1], _F32, kind="ExternalInput")
    # y rows are padded to 4096 cols so each store row is a single 8KB
    # DMA packet (8188-byte rows split into two ~4KB packets, halving the
    # per-stream DMA rate). Host slices off the 2 pad columns.
    y = nc.dram_tensor("y", [N_FULL_PANELS * PANEL_OUT, W], _BF16,
                       kind="ExternalOutput")
    yt = nc.dram_tensor("yt", [TJ * TAIL_OUT, TN], _BF16,
                        kind="ExternalOutput")

    TK = TJ * TAIL_IN   # 120
    TM = TJ * TAIL_OUT  # 96

    with tile.TileContext(nc) as tc:
        with (
            tc.tile_pool(name="const", bufs=1) as cpool,
            tc.tile_pool(name="xp", bufs=5) as xpool,
            tc.tile_pool(name="op", bufs=3) as opool,
            tc.tile_pool(name="pp", bufs=4, space="PSUM") as ppool,
        ):
            # Warmup memset first on gpsimd (its queue only carries the
            # stores, pushed much later), so the PE can start at once.
            wz = cpool.tile([128, COLS_PER_MM], _BF16)
            nc.gpsimd.memset(wz[:], 0.0)

            # All loads ride the sync queue as full-width DMAs: 4096 bf16
            # cols = one 8KB packet per partition row, the shape that
            # sustains the full ~430 GB/s. Panel 0 goes first; the small
            # constants follow it so the first matmul's weights are ready
            # well before panel 0 completes.
            # Sync carries ONLY the four big panel loads; every small
            # tensor rides the scalar queue so its ~500 sub-1KB packets
            # don't steal packet slots from panel 0 in the HAM-capped
            # early window.
            xts = []
            for panel in range(N_FULL_PANELS):
                xt = xpool.tile([128, W], _BF16)
                xts.append(xt)
            nc.sync.dma_start(xts[0][:], x[0:128, :])
            wt = cpool.tile([128, KW * PANEL_OUT], _BF16)
            nc.scalar.dma_start(wt[:], w[:])
            bt = cpool.tile([128, 1], _F32)
            nc.scalar.dma_start(bt[:], b[:])
            wtt = cpool.tile([TK, KW * TM], _BF16)
            nc.scalar.dma_start(wtt[:], wt_p[:])
            xtt = cpool.tile([TK, TN + KW - 1], _BF16)
            nc.scalar.dma_start(xtt[:], xt_p[:])
            for panel in range(1, N_FULL_PANELS):
                r0 = PANEL_OUT * panel
                nc.sync.dma_start(xts[panel][:], x[r0 : r0 + 128, :])

            # PE warmup on zeroed tiles: keeps the PE busy (DVFS ramping)
            # while panel 0 streams in.
            psw = ppool.tile([128, CHUNK], _F32, tag="ps")
            for _ in range(WARMUP_MM):
                nc.tensor.matmul(
                    psw[:126, :COLS_PER_MM],
                    wz[:, :126],
                    wz[:, :],
                    start=True,
                    stop=True,
                )

            # Packed tail: one 3-matmul group covers all 8 tail rows.
            pst = ppool.tile([128, CHUNK], _F32, tag="ps")
            for dc in range(KW):
                nc.tensor.matmul(
                    pst[:TM, :TN],
                    wtt[:TK, dc * TM : dc * TM + TM],
                    xtt[:TK, dc : dc + TN],
                    start=(dc == 0),
                    stop=(dc == KW - 1),
                )
            ott = opool.tile([TM, TN], _BF16)
            nc.scalar.activation(
                ott[:TM, :TN],
                pst[:TM, :TN],
                mybir.ActivationFunctionType.Identity,
                bias=bt[:TM, :],
            )
            nc.gpsimd.dma_start(yt[:, :], ott[:TM, :TN])

            def do_panel(panel):
                r0 = PANEL_OUT * panel
                xt = xts[panel]
                ot = opool.tile([128, W], _BF16)
                # Pad columns 4094:4096 so the full 8KB store row is
                # initialized (values are ignored by the host).
                nc.vector.memset(ot[:PANEL_OUT, OW:W], 0.0)
                for c in range(4):
                    ps = ppool.tile([128, CHUNK], _F32, tag="ps")
                    s0 = c * CHUNK
                    sw = min(CHUNK, OW - s0)  # 1024 / 1022
                    for dc in range(KW):
                        for jj in range(2):
                            c0 = s0 + jj * COLS_PER_MM
                            N = min(COLS_PER_MM, OW - c0)
                            lc0 = jj * COLS_PER_MM
                            nc.tensor.matmul(
                                ps[:PANEL_OUT, lc0 : lc0 + N],
                                wt[:128, dc * PANEL_OUT : dc * PANEL_OUT + PANEL_OUT],
                                xt[:128, c0 + dc : c0 + dc + N],
                                start=(dc == 0),
                                stop=(dc == KW - 1),
                            )
                    # Drain PSUM on alternating engines so neither ScalarE
                    # nor VectorE becomes the bottleneck; bias is fused.
                    if c % 2 == 0:
                        nc.scalar.activation(
                            ot[:PANEL_OUT, s0 : s0 + sw],
                            ps[:PANEL_OUT, :sw],
                            mybir.ActivationFunctionType.Identity,
                            bias=bt[:PANEL_OUT, :],
                        )
                    else:
                        nc.vector.tensor_scalar_add(
                            ot[:PANEL_OUT, s0 : s0 + sw],
                            ps[:PANEL_OUT, :sw],
                            bt[:PANEL_OUT, :],
                        )
                # One full-width 8KB-packet store per panel, alternating
                # queues (a single queue sustains only ~8 write streams).
                eng = nc.gpsimd if panel % 2 == 0 else nc.sync
                eng.dma_start(y[r0 : r0 + PANEL_OUT, :], ot[:PANEL_OUT, :W])

            for panel in range(N_FULL_PANELS):
                do_panel(panel)

    nc.compile()
    return nc


def _banded_weights(weight: np.ndarray) -> np.ndarray:
    """lhsT for each kernel column dc, laid out as [128, KW*PANEL_OUT].

    wT[k, dc*PANEL_OUT + m] = weight[k - m, dc] for 0 <= k - m < KH.
    """
    wT = np.zeros((128, KW * PANEL_OUT), np.float32)
    m = np.arange(PANEL_OUT)
    for dc in range(KW):
        for d in range(KH):
            wT[m + d, dc * PANEL_OUT + m] = weight[d, dc]
    return wT.astype(BF)


def _tail_weights(weight: np.ndarray) -> np.ndarray:
    """Block-diagonal banded stationary for the packed tail.

    S[10j + m + d, dc*96 + 8j + m] = weight[d, dc].
    """
    TK = TJ * TAIL_IN
    TM = TJ * TAIL_OUT
    S = np.zeros((TK, KW * TM), np.float32)
    m = np.arange(TAIL_OUT)
    for dc in range(KW):
        for j in range(TJ):
            for d in range(KH):
                S[TAIL_IN * j + m + d, dc * TM + TAIL_OUT * j + m] = weight[d, dc]
    return S.astype(BF)


def _install_ntff_hook():
    """Shim antenv.axon_hooks so run_bass_kernel_spmd(trace=True) can find
    the axon NTFF profiling hook (the image's antenv lacks axon_hooks)."""
    import sys
    import types

    try:
        from antenv.axon_hooks import get_axon_ntff_profile_hook  # noqa: F401

        return
    except ImportError:
        pass
    import antenv
    from trn_agent_boot.trn_boot import _ntff_profile_via_ctypes

    hook = _ntff_profile_via_ctypes("/opt/axon/libaxon_pjrt.so")
    mod = types.ModuleType("antenv.axon_hooks")
    mod._hook = hook
    mod.set_axon_ntff_profile_hook = lambda h: setattr(mod, "_hook", h)
    mod.get_axon_ntff_profile_hook = lambda: mod._hook
    sys.modules["antenv.axon_hooks"] = mod
    antenv.axon_hooks = mod


def kernel(x, weight, bias, _trace=False, _trace_cores=None):
    global _PROGRAM_CACHE, last_results
    if _trace:
        _install_ntff_hook()
    x = np.asarray(x, dtype=np.float32)
    weight = np.asarray(weight, dtype=np.float32)
    bias = np.asarray(bias, dtype=np.float32)

    if _PROGRAM_CACHE is None:
        _PROGRAM_CACHE = _build_program()
    nc = _PROGRAM_CACHE

    xbf = x.astype(BF)
    wT = _banded_weights(weight)
    wtail = _tail_weights(weight)
    bb = np.full((128, 1), bias[0], np.float32)

    in_maps = []
    for i in range(NCORES):
        r0 = i * ROWS_PER_CORE if i < NCORES - 1 else H - IN_ROWS
        xc = xbf[r0 : r0 + IN_ROWS]
        # Packed tail moving operand: partition 10j+i = tail input row i,
        # column block j (stride TSTRIDE, width TN+2).
        tr = xc[N_FULL_PANELS * PANEL_OUT :]  # rows 504..514
        xtp = np.stack(
            [tr[:, TSTRIDE * j : TSTRIDE * j + TN + KW - 1] for j in range(TJ)]
        ).reshape(TJ * TAIL_IN, TN + KW - 1)
        in_maps.append(
            {
                "x": np.ascontiguousarray(xc),
                "xt": np.ascontiguousarray(xtp),
                "w": wT,
                "wt": wtail,
                "b": bb,
            }
        )

    kwargs = {}
    if _trace:
        kwargs["trace"] = True
        kwargs["trace_cores"] = (
            list(range(NCORES)) if _trace_cores is None else _trace_cores
        )
    res = run_bass_kernel_spmd(nc, in_maps, core_ids=list(range(NCORES)), **kwargs)
    last_results = res

    out = np.empty((OH, OW), np.float32)
    for i in range(NCORES):
        r0 = i * ROWS_PER_CORE if i < NCORES - 1 else H - IN_ROWS
        yi = res.results[i]["y"][:, :OW].astype(np.float32)
        out[r0 : r0 + N_FULL_PANELS * PANEL_OUT] = yi
        # Unpack the packed tail: partition 8j+m = tail row m, col block j.
        yti = res.results[i]["yt"].astype(np.float32)
        for j in range(TJ):
            out[
                r0 + N_FULL_PANELS * PANEL_OUT : r0 + ROWS_PER_CORE,
                TSTRIDE * j : TSTRIDE * j + TN,
            ] = yti[TAIL_OUT * j : TAIL_OUT * (j + 1)]
    return out


# revision 29
# speedup vs baseline: 1.1891x; 1.1891x over previous
"""3x3 valid cross-correlation of a 4096x4096 fp32 image + scalar bias,
sharded row-wise across 8 TRN2 NeuronCores.

bf16 datapath (harness gate is rel_err < 2e-2; bf16 lands ~5e-3):
  - x is cast to bf16 on host -> load DMA traffic halves (4.2 MB/core).
  - Matmuls run bf16 x bf16 -> fp32 PSUM at 1 cycle/column.
  - Output is stored as bf16 (4.2 MB/core) and upcast to fp32 on host.

Strategy per core (512 output rows, 514 input rows incl. 2-row halo taken
host-side via overlapping slices -- no device collectives):
  - Row panels of 128 input rows -> 126 output rows (banded matmul):
    out[m, n] = sum_dc sum_dr w[dr, dc] * x[m+dr, n+dc]
    For each kernel column dc, a banded stationary matrix
    B_dc[k, m] = w[k-m, dc] (k-m in 0..2) gives
    psum[m, n] += sum_k B_dc[k, m] * x[k, n+dc].
  - The 8-row tail (rows 504..512) is packed 12 column-blocks deep into
    the PE contraction dim: stationary [120, 96] block-diagonal banded
    matrix, moving operand [120, 345] gathered on host so partition
    10j+i = x[504+i, 341j:341j+345]. 3 matmuls of 343 columns replace a
    full 3x4094-column pass (12x fewer PE cycles for the tail).
  - Scheduling is dominated by the NC activity manager (HAM): DMA is
    capped at ~230-270 GB/s and the PE at ~1.2 GHz until ~5-6us of
    sustained activity earn the full-rate grant (~430 GB/s, 2.4 GHz),
    and any PE idle gap early in the run triggers a half-rate (k=4/8)
    throttle spiral. The warmup matmuls therefore bridge the PE from
    its first possible cycle (~8us, after the fixed SPMD prologue)
    until panel 0's load semaphore fires (~13-14.5us), so the matmul
    stream runs gap-free at full clock (215ns per 512-col matmul).
  - Bias is fused into the PSUM->SBUF drain (ScalarE activation bias for
    even chunks, VectorE tensor_scalar_add for odd ones, both converting
    to bf16).
  - Store rows are padded to 4096 cols (single 8KB packet per row) and
    panels alternate between the gpsimd and sync queues: one queue
    sustains only ~8 concurrent write streams (~200 GB/s).
  - Last core overlaps core 6 by 2 rows so all cores run an identical
    514-row program (4094 = 8*512 - 2).
"""

import numpy as np
import ml_dtypes

import concourse.bacc as bacc
import concourse.mybir as mybir
from concourse import tile
from concourse.bass_utils import run_bass_kernel_spmd

H, W = 4096, 4096
KH, KW = 3, 3
OH, OW = H - KH + 1, W - KW + 1  # 4094, 4094
NCORES = 8
ROWS_PER_CORE = 512              # output rows computed per core
IN_ROWS = ROWS_PER_CORE + KH - 1  # 514 input rows per core
PANEL_OUT = 126                  # output rows per full 128-input-row panel
N_FULL_PANELS = 4                # 4 * 126 = 504
TAIL_OUT = ROWS_PER_CORE - N_FULL_PANELS * PANEL_OUT  # 8
TAIL_IN = TAIL_OUT + KH - 1      # 10
COLS_PER_MM = 512                # PSUM-bank max (512 fp32)
CHUNK = 1024                     # PSUM chunk = 2 banks
# Packed tail geometry: 12 column blocks, stride 341, 343 output columns
# each; 341*11 + 343 = 4094 exactly, and input reads stop at 4096.
TJ = 12
TSTRIDE = 341
TN = 343
WARMUP_MM = 15

_F32 = mybir.dt.float32
_BF16 = mybir.dt.bfloat16
BF = ml_dtypes.bfloat16

_PROGRAM_CACHE = None
last_results = None  # BassKernelResults of the most recent kernel() call


def _build_program():
    nc = bacc.Bacc(
        "TRN2", target_bir_lowering=False, debug=False, num_devices=NCORES
    )
    x = nc.dram_tensor("x", [IN_ROWS, W], _BF16, kind="ExternalInput")
    xt_p = nc.dram_tensor("xt", [TJ * TAIL_IN, TN + KW - 1], _BF16,
                          kind="ExternalInput")
    w = nc.dram_tensor("w", [128, KW * PANEL_OUT], _BF16, kind="ExternalInput")
    wt_p = nc.dram_tensor("wt", [TJ * TAIL_IN, KW * TJ * TAIL_OUT], _BF16,
                          kind="ExternalInput")
    b = nc.dram_tensor("b", [128, 1], _F32, kind="ExternalInput")
    # y rows are padded to 4096 cols so each store row is a single 8KB
    # DMA packet (8188-byte rows split into two ~4KB packets, halving the
    # per-stream DMA rate). Host slices off the 2 pad columns.
    y = nc.dram_tensor("y", [N_FULL_PANELS * PANEL_OUT, W], _BF16,
                       kind="ExternalOutput")
    yt = nc.dram_tensor("yt", [TJ * TAIL_OUT, TN], _BF16,
                        kind="ExternalOutput")

    TK = TJ * TAIL_IN   # 120
    TM = TJ * TAIL_OUT  # 96

    with tile.TileContext(nc) as tc:
        with (
            tc.tile_pool(name="const", bufs=1) as cpool,
            tc.tile_pool(name="xp", bufs=5) as xpool,
            tc.tile_pool(name="op", bufs=3) as opool,
            tc.tile_pool(name="pp", bufs=4, space="PSUM") as ppool,
        ):
            # Warmup memset first on gpsimd (its queue only carries the
            # stores, pushed much later), so the PE can start at once.
            wz = cpool.tile([128, COLS_PER_MM], _BF16)
            nc.gpsimd.memset(wz[:], 0.0)

            # All loads ride the sync queue as full-width DMAs: 4096 bf16
            # cols = one 8KB packet per partition row, the shape that
            # sustains the full ~430 GB/s. Panel 0 goes first; the small
            # constants follow it so the first matmul's weights are ready
            # well before panel 0 completes.
            xts = []
            for panel in range(N_FULL_PANELS):
                xt = xpool.tile([128, W], _BF16)
                xts.append(xt)
            nc.sync.dma_start(xts[0][:], x[0:128, :])
            wt = cpool.tile([128, KW * PANEL_OUT], _BF16)
            nc.sync.dma_start(wt[:], w[:])
            bt = cpool.tile([128, 1], _F32)
            nc.sync.dma_start(bt[:], b[:])
            wtt = cpool.tile([TK, KW * TM], _BF16)
            nc.sync.dma_start(wtt[:], wt_p[:])
            for panel in range(1, N_FULL_PANELS):
                r0 = PANEL_OUT * panel
                nc.sync.dma_start(xts[panel][:], x[r0 : r0 + 128, :])
            xtt = cpool.tile([TK, TN + KW - 1], _BF16)
            nc.sync.dma_start(xtt[:], xt_p[:])

            # PE warmup on zeroed tiles: keeps the PE busy (DVFS ramping)
            # while panel 0 streams in.
            psw = ppool.tile([128, CHUNK], _F32, tag="ps")
            for _ in range(WARMUP_MM):
                nc.tensor.matmul(
                    psw[:126, :COLS_PER_MM],
                    wz[:, :126],
                    wz[:, :],
                    start=True,
                    stop=True,
                )

            def do_panel(panel):
                r0 = PANEL_OUT * panel
                xt = xts[panel]
                ot = opool.tile([128, W], _BF16)
                # Pad columns 4094:4096 so the full 8KB store row is
                # initialized (values are ignored by the host).
                nc.vector.memset(ot[:PANEL_OUT, OW:W], 0.0)
                for c in range(4):
                    ps = ppool.tile([128, CHUNK], _F32, tag="ps")
                    s0 = c * CHUNK
                    sw = min(CHUNK, OW - s0)  # 1024 / 1022
                    for dc in range(KW):
                        for jj in range(2):
                            c0 = s0 + jj * COLS_PER_MM
                            N = min(COLS_PER_MM, OW - c0)
                            lc0 = jj * COLS_PER_MM
                            nc.tensor.matmul(
                                ps[:PANEL_OUT, lc0 : lc0 + N],
                                wt[:128, dc * PANEL_OUT : dc * PANEL_OUT + PANEL_OUT],
                                xt[:128, c0 + dc : c0 + dc + N],
                                start=(dc == 0),
                                stop=(dc == KW - 1),
                            )
                    # Drain PSUM on alternating engines so neither ScalarE
                    # nor VectorE becomes the bottleneck; bias is fused.
                    if c % 2 == 0:
                        nc.scalar.activation(
                            ot[:PANEL_OUT, s0 : s0 + sw],
                            ps[:PANEL_OUT, :sw],
                            mybir.ActivationFunctionType.Identity,
                            bias=bt[:PANEL_OUT, :],
                        )
                    else:
                        nc.vector.tensor_scalar_add(
                            ot[:PANEL_OUT, s0 : s0 + sw],
                            ps[:PANEL_OUT, :sw],
                            bt[:PANEL_OUT, :],
                        )
                # One full-width 8KB-packet store per panel, alternating
                # queues (a single queue sustains only ~8 write streams).
                eng = nc.gpsimd if panel % 2 == 0 else nc.sync
                eng.dma_start(y[r0 : r0 + PANEL_OUT, :], ot[:PANEL_OUT, :W])

            for panel in range(N_FULL_PANELS):
                do_panel(panel)
            # Packed tail: one 3-matmul group covers all 8 tail rows.
            pst = ppool.tile([128, CHUNK], _F32, tag="ps")
            for dc in range(KW):
                nc.tensor.matmul(
                    pst[:TM, :TN],
                    wtt[:TK, dc * TM : dc * TM + TM],
                    xtt[:TK, dc : dc + TN],
                    start=(dc == 0),
                    stop=(dc == KW - 1),
                )
            ott = opool.tile([TM, TN], _BF16)
            nc.scalar.activation(
                ott[:TM, :TN],
                pst[:TM, :TN],
                mybir.ActivationFunctionType.Identity,
                bias=bt[:TM, :],
            )
            nc.gpsimd.dma_start(yt[:, :], ott[:TM, :TN])

    nc.compile()
    return nc


def _banded_weights(weight: np.ndarray) -> np.ndarray:
    """lhsT for each kernel column dc, laid out as [128, KW*PANEL_OUT].

    wT[k, dc*PANEL_OUT + m] = weight[k - m, dc] for 0 <= k - m < KH.
    """
    wT = np.zeros((128, KW * PANEL_OUT), np.float32)
    m = np.arange(PANEL_OUT)
    for dc in range(KW):
        for d in range(KH):
            wT[m + d, dc * PANEL_OUT + m] = weight[d, dc]
    return wT.astype(BF)


def _tail_weights(weight: np.ndarray) -> np.ndarray:
    """Block-diagonal banded stationary for the packed tail.

    S[10j + m + d, dc*96 + 8j + m] = weight[d, dc].
    """
    TK = TJ * TAIL_IN
    TM = TJ * TAIL_OUT
    S = np.zeros((TK, KW * TM), np.float32)
    m = np.arange(TAIL_OUT)
    for dc in range(KW):
        for j in range(TJ):
            for d in range(KH):
                S[TAIL_IN * j + m + d, dc * TM + TAIL_OUT * j + m] = weight[d, dc]
    return S.astype(BF)


def _install_ntff_hook():
    """Shim antenv.axon_hooks so run_bass_kernel_spmd(trace=True) can find
    the axon NTFF profiling hook (the image's antenv lacks axon_hooks)."""
    import sys
    import types

    try:
        from antenv.axon_hooks import get_axon_ntff_profile_hook  # noqa: F401

        return
    except ImportError:
        pass
    import antenv
    from trn_agent_boot.trn_boot import _ntff_profile_via_ctypes

    hook = _ntff_profile_via_ctypes("/opt/axon/libaxon_pjrt.so")
    mod = types.ModuleType("antenv.axon_hooks")
    mod._hook = hook
    mod.set_axon_ntff_profile_hook = lambda h: setattr(mod, "_hook", h)
    mod.get_axon_ntff_profile_hook = lambda: mod._hook
    sys.modules["antenv.axon_hooks"] = mod
    antenv.axon_hooks = mod


def kernel(x, weight, bias, _trace=False, _trace_cores=None):
    global _PROGRAM_CACHE, last_results
    if _trace:
        _install_ntff_hook()
    x = np.asarray(x, dtype=np.float32)
    weight = np.asarray(weight, dtype=np.float32)
    bias = np.asarray(bias, dtype=np.float32)

    if _PROGRAM_CACHE is None:
        _PROGRAM_CACHE = _build_program()
    nc = _PROGRAM_CACHE

    xbf = x.astype(BF)
    wT = _banded_weights(weight)
    wtail = _tail_weights(weight)
    bb = np.full((128, 1), bias[0], np.float32)

    in_maps = []
    for i in range(NCORES):
        r0 = i * ROWS_PER_CORE if i < NCORES - 1 else H - IN_ROWS
        xc = xbf[r0 : r0 + IN_ROWS]
        # Packed tail moving operand: partition 10j+i = tail input row i,
        # column block j (stride TSTRIDE, width TN+2).
        tr = xc[N_FULL_PANELS * PANEL_OUT :]  # rows 504..514
        xtp = np.stack(
            [tr[:, TSTRIDE * j : TSTRIDE * j + TN + KW - 1] for j in range(TJ)]
        ).reshape(TJ * TAIL_IN, TN + KW - 1)
        in_maps.append(
            {
                "x": np.ascontiguousarray(xc),
                "xt": np.ascontiguousarray(xtp),
                "w": wT,
                "wt": wtail,
                "b": bb,
            }
        )

    kwargs = {}
    if _trace:
        kwargs["trace"] = True
        kwargs["trace_cores"] = (
            list(range(NCORES)) if _trace_cores is None else _trace_cores
        )
    res = run_bass_kernel_spmd(nc, in_maps, core_ids=list(range(NCORES)), **kwargs)
    last_results = res

    out = np.empty((OH, OW), np.float32)
    for i in range(NCORES):
        r0 = i * ROWS_PER_CORE if i < NCORES - 1 else H - IN_ROWS
        yi = res.results[i]["y"][:, :OW].astype(np.float32)
        out[r0 : r0 + N_FULL_PANELS * PANEL_OUT] = yi
        # Unpack the packed tail: partition 8j+m = tail row m, col block j.
        yti = res.results[i]["yt"].astype(np.float32)
        for j in range(TJ):
            out[
                r0 + N_FULL_PANELS * PANEL_OUT : r0 + ROWS_PER_CORE,
                TSTRIDE * j : TSTRIDE * j + TN,
            ] = yti[TAIL_OUT * j : TAIL_OUT * (j + 1)]
    return out


# revision 30
# speedup vs baseline: 1.2270x; 1.0319x over previous
"""3x3 valid cross-correlation of a 4096x4096 fp32 image + scalar bias,
sharded row-wise across 8 TRN2 NeuronCores.

bf16 datapath (harness gate is rel_err < 2e-2; bf16 lands ~5e-3):
  - x is cast to bf16 on host -> load DMA traffic halves (4.2 MB/core).
  - Matmuls run bf16 x bf16 -> fp32 PSUM at 1 cycle/column.
  - Output is stored as bf16 (4.2 MB/core) and upcast to fp32 on host.

Strategy per core (512 output rows, 514 input rows incl. 2-row halo taken
host-side via overlapping slices -- no device collectives):
  - Row panels of 128 input rows -> 126 output rows (banded matmul):
    out[m, n] = sum_dc sum_dr w[dr, dc] * x[m+dr, n+dc]
    For each kernel column dc, a banded stationary matrix
    B_dc[k, m] = w[k-m, dc] (k-m in 0..2) gives
    psum[m, n] += sum_k B_dc[k, m] * x[k, n+dc].
  - The 8-row tail (rows 504..512) is packed 12 column-blocks deep into
    the PE contraction dim: stationary [120, 96] block-diagonal banded
    matrix, moving operand [120, 345] gathered on host so partition
    10j+i = x[504+i, 341j:341j+345]. 3 matmuls of 343 columns replace a
    full 3x4094-column pass (12x fewer PE cycles for the tail).
  - Scheduling is dominated by the NC activity manager (HAM): DMA is
    capped at ~230-270 GB/s and the PE at ~1.2 GHz until ~5-6us of
    sustained activity earn the full-rate grant (~430 GB/s, 2.4 GHz),
    and any PE idle gap early in the run triggers a half-rate (k=4/8)
    throttle spiral. The warmup matmuls therefore bridge the PE from
    its first possible cycle (~8us, after the fixed SPMD prologue)
    until panel 0's load semaphore fires (~13-14.5us), so the matmul
    stream runs gap-free at full clock (215ns per 512-col matmul).
  - Bias is fused into the PSUM->SBUF drain (ScalarE activation bias for
    even chunks, VectorE tensor_scalar_add for odd ones, both converting
    to bf16).
  - Store rows are padded to 4096 cols (single 8KB packet per row) and
    panels alternate between the gpsimd and sync queues: one queue
    sustains only ~8 concurrent write streams (~200 GB/s).
  - Last core overlaps core 6 by 2 rows so all cores run an identical
    514-row program (4094 = 8*512 - 2).
"""

import numpy as np
import ml_dtypes

import concourse.bacc as bacc
import concourse.mybir as mybir
from concourse import tile
from concourse.bass_utils import run_bass_kernel_spmd

H, W = 4096, 4096
KH, KW = 3, 3
OH, OW = H - KH + 1, W - KW + 1  # 4094, 4094
NCORES = 8
ROWS_PER_CORE = 512              # output rows computed per core
IN_ROWS = ROWS_PER_CORE + KH - 1  # 514 input rows per core
PANEL_OUT = 126                  # output rows per full 128-input-row panel
N_FULL_PANELS = 4                # 4 * 126 = 504
TAIL_OUT = ROWS_PER_CORE - N_FULL_PANELS * PANEL_OUT  # 8
TAIL_IN = TAIL_OUT + KH - 1      # 10
COLS_PER_MM = 512                # PSUM-bank max (512 fp32)
CHUNK = 1024                     # PSUM chunk = 2 banks
# Packed tail geometry: 12 column blocks, stride 341, 343 output columns
# each; 341*11 + 343 = 4094 exactly, and input reads stop at 4096.
TJ = 12
TSTRIDE = 341
TN = 343
WARMUP_MM = 15

_F32 = mybir.dt.float32
_BF16 = mybir.dt.bfloat16
BF = ml_dtypes.bfloat16

_PROGRAM_CACHE = None
last_results = None  # BassKernelResults of the most recent kernel() call


def _build_program():
    nc = bacc.Bacc(
        "TRN2", target_bir_lowering=False, debug=False, num_devices=NCORES
    )
    x = nc.dram_tensor("x", [IN_ROWS, W], _BF16, kind="ExternalInput")
    xt_p = nc.dram_tensor("xt", [TJ * TAIL_IN, TN + KW - 1], _BF16,
                          kind="ExternalInput")
    w = nc.dram_tensor("w", [128, KW * PANEL_OUT], _BF16, kind="ExternalInput")
    wt_p = nc.dram_tensor("wt", [TJ * TAIL_IN, KW * TJ * TAIL_OUT], _BF16,
                          kind="ExternalInput")
    b = nc.dram_tensor("b", [128, 1], _F32, kind="ExternalInput")
    # y rows are padded to 4096 cols so each store row is a single 8KB
    # DMA packet (8188-byte rows split into two ~4KB packets, halving the
    # per-stream DMA rate). Host slices off the 2 pad columns.
    y = nc.dram_tensor("y", [N_FULL_PANELS * PANEL_OUT, W], _BF16,
                       kind="ExternalOutput")
    yt = nc.dram_tensor("yt", [TJ * TAIL_OUT, TN], _BF16,
                        kind="ExternalOutput")

    TK = TJ * TAIL_IN   # 120
    TM = TJ * TAIL_OUT  # 96

    with tile.TileContext(nc) as tc:
        with (
            tc.tile_pool(name="const", bufs=1) as cpool,
            tc.tile_pool(name="xp", bufs=5) as xpool,
            tc.tile_pool(name="op", bufs=3) as opool,
            tc.tile_pool(name="pp", bufs=4, space="PSUM") as ppool,
        ):
            # Warmup memset first on gpsimd (its queue only carries the
            # stores, pushed much later), so the PE can start at once.
            wz = cpool.tile([128, COLS_PER_MM], _BF16)
            nc.gpsimd.memset(wz[:], 0.0)

            # All loads ride the sync queue as full-width DMAs: 4096 bf16
            # cols = one 8KB packet per partition row, the shape that
            # sustains the full ~430 GB/s. Panel 0 goes first; the small
            # constants follow it so the first matmul's weights are ready
            # well before panel 0 completes.
            xts = []
            for panel in range(N_FULL_PANELS):
                xt = xpool.tile([128, W], _BF16)
                xts.append(xt)
            nc.sync.dma_start(xts[0][:], x[0:128, :])
            wt = cpool.tile([128, KW * PANEL_OUT], _BF16)
            nc.sync.dma_start(wt[:], w[:])
            bt = cpool.tile([128, 1], _F32)
            nc.sync.dma_start(bt[:], b[:])
            wtt = cpool.tile([TK, KW * TM], _BF16)
            nc.sync.dma_start(wtt[:], wt_p[:])
            for panel in range(1, N_FULL_PANELS):
                r0 = PANEL_OUT * panel
                nc.sync.dma_start(xts[panel][:], x[r0 : r0 + 128, :])
            xtt = cpool.tile([TK, TN + KW - 1], _BF16)
            nc.sync.dma_start(xtt[:], xt_p[:])

            # PE warmup on zeroed tiles: keeps the PE busy (DVFS ramping)
            # while panel 0 streams in.
            psw = ppool.tile([128, CHUNK], _F32, tag="ps")
            for _ in range(WARMUP_MM):
                nc.tensor.matmul(
                    psw[:126, :COLS_PER_MM],
                    wz[:, :126],
                    wz[:, :],
                    start=True,
                    stop=True,
                )

            def do_panel(panel):
                r0 = PANEL_OUT * panel
                xt = xts[panel]
                ot = opool.tile([128, W], _BF16)
                # Pad columns 4094:4096 so the full 8KB store row is
                # initialized (values are ignored by the host).
                nc.vector.memset(ot[:PANEL_OUT, OW:W], 0.0)
                for c in range(4):
                    ps = ppool.tile([128, CHUNK], _F32, tag="ps")
                    s0 = c * CHUNK
                    sw = min(CHUNK, OW - s0)  # 1024 / 1022
                    for dc in range(KW):
                        for jj in range(2):
                            c0 = s0 + jj * COLS_PER_MM
                            N = min(COLS_PER_MM, OW - c0)
                            lc0 = jj * COLS_PER_MM
                            nc.tensor.matmul(
                                ps[:PANEL_OUT, lc0 : lc0 + N],
                                wt[:128, dc * PANEL_OUT : dc * PANEL_OUT + PANEL_OUT],
                                xt[:128, c0 + dc : c0 + dc + N],
                                start=(dc == 0),
                                stop=(dc == KW - 1),
                            )
                    # Drain PSUM on alternating engines so neither ScalarE
                    # nor VectorE becomes the bottleneck; bias is fused.
                    if c % 2 == 0:
                        nc.scalar.activation(
                            ot[:PANEL_OUT, s0 : s0 + sw],
                            ps[:PANEL_OUT, :sw],
                            mybir.ActivationFunctionType.Identity,
                            bias=bt[:PANEL_OUT, :],
                        )
                    else:
                        nc.vector.tensor_scalar_add(
                            ot[:PANEL_OUT, s0 : s0 + sw],
                            ps[:PANEL_OUT, :sw],
                            bt[:PANEL_OUT, :],
                        )
                # One full-width 8KB-packet store per panel, alternating
                # queues (a single queue sustains only ~8 write streams).
                # The last panel ends the critical path, so it stores as
                # two row halves on both queues at once (loads are long
                # done by then, so no queue interference).
                if panel < N_FULL_PANELS - 1:
                    eng = nc.gpsimd if panel % 2 == 0 else nc.sync
                    eng.dma_start(y[r0 : r0 + PANEL_OUT, :], ot[:PANEL_OUT, :W])
                else:
                    nc.sync.dma_start(y[r0 : r0 + 63, :], ot[:63, :W])
                    nc.gpsimd.dma_start(
                        y[r0 + 63 : r0 + PANEL_OUT, :], ot[63:PANEL_OUT, :W]
                    )

            for panel in range(N_FULL_PANELS):
                do_panel(panel)
            # Packed tail: one 3-matmul group covers all 8 tail rows.
            pst = ppool.tile([128, CHUNK], _F32, tag="ps")
            for dc in range(KW):
                nc.tensor.matmul(
                    pst[:TM, :TN],
                    wtt[:TK, dc * TM : dc * TM + TM],
                    xtt[:TK, dc : dc + TN],
                    start=(dc == 0),
                    stop=(dc == KW - 1),
                )
            ott = opool.tile([TM, TN], _BF16)
            nc.scalar.activation(
                ott[:TM, :TN],
                pst[:TM, :TN],
                mybir.ActivationFunctionType.Identity,
                bias=bt[:TM, :],
            )
            nc.gpsimd.dma_start(yt[:, :], ott[:TM, :TN])

    nc.compile()
    return nc


def _banded_weights(weight: np.ndarray) -> np.ndarray:
    """lhsT for each kernel column dc, laid out as [128, KW*PANEL_OUT].

    wT[k, dc*PANEL_OUT + m] = weight[k - m, dc] for 0 <= k - m < KH.
    """
    wT = np.zeros((128, KW * PANEL_OUT), np.float32)
    m = np.arange(PANEL_OUT)
    for dc in range(KW):
        for d in range(KH):
            wT[m + d, dc * PANEL_OUT + m] = weight[d, dc]
    return wT.astype(BF)


def _tail_weights(weight: np.ndarray) -> np.ndarray:
    """Block-diagonal banded stationary for the packed tail.

    S[10j + m + d, dc*96 + 8j + m] = weight[d, dc].
    """
    TK = TJ * TAIL_IN
    TM = TJ * TAIL_OUT
    S = np.zeros((TK, KW * TM), np.float32)
    m = np.arange(TAIL_OUT)
    for dc in range(KW):
        for j in range(TJ):
            for d in range(KH):
                S[TAIL_IN * j + m + d, dc * TM + TAIL_OUT * j + m] = weight[d, dc]
    return S.astype(BF)


def _install_ntff_hook():
    """Shim antenv.axon_hooks so run_bass_kernel_spmd(trace=True) can find
    the axon NTFF profiling hook (the image's antenv lacks axon_hooks)."""
    import sys
    import types

    try:
        from antenv.axon_hooks import get_axon_ntff_profile_hook  # noqa: F401

        return
    except ImportError:
        pass
    import antenv
    from trn_agent_boot.trn_boot import _ntff_profile_via_ctypes

    hook = _ntff_profile_via_ctypes("/opt/axon/libaxon_pjrt.so")
    mod = types.ModuleType("antenv.axon_hooks")
    mod._hook = hook
    mod.set_axon_ntff_profile_hook = lambda h: setattr(mod, "_hook", h)
    mod.get_axon_ntff_profile_hook = lambda: mod._hook
    sys.modules["antenv.axon_hooks"] = mod
    antenv.axon_hooks = mod


def kernel(x, weight, bias, _trace=False, _trace_cores=None):
    global _PROGRAM_CACHE, last_results
    if _trace:
        _install_ntff_hook()
    x = np.asarray(x, dtype=np.float32)
    weight = np.asarray(weight, dtype=np.float32)
    bias = np.asarray(bias, dtype=np.float32)

    if _PROGRAM_CACHE is None:
        _PROGRAM_CACHE = _build_program()
    nc = _PROGRAM_CACHE

    xbf = x.astype(BF)
    wT = _banded_weights(weight)
    wtail = _tail_weights(weight)
    bb = np.full((128, 1), bias[0], np.float32)

    in_maps = []
    for i in range(NCORES):
        r0 = i * ROWS_PER_CORE if i < NCORES - 1 else H - IN_ROWS
        xc = xbf[r0 : r0 + IN_ROWS]
        # Packed tail moving operand: partition 10j+i = tail input row i,
        # column block j (stride TSTRIDE, width TN+2).
        tr = xc[N_FULL_PANELS * PANEL_OUT :]  # rows 504..514
        xtp = np.stack(
            [tr[:, TSTRIDE * j : TSTRIDE * j + TN + KW - 1] for j in range(TJ)]
        ).reshape(TJ * TAIL_IN, TN + KW - 1)
        in_maps.append(
            {
                "x": np.ascontiguousarray(xc),
                "xt": np.ascontiguousarray(xtp),
                "w": wT,
                "wt": wtail,
                "b": bb,
            }
        )

    kwargs = {}
    if _trace:
        kwargs["trace"] = True
        kwargs["trace_cores"] = (
            list(range(NCORES)) if _trace_cores is None else _trace_cores
        )
    res = run_bass_kernel_spmd(nc, in_maps, core_ids=list(range(NCORES)), **kwargs)
    last_results = res

    out = np.empty((OH, OW), np.float32)
    for i in range(NCORES):
        r0 = i * ROWS_PER_CORE if i < NCORES - 1 else H - IN_ROWS
        yi = res.results[i]["y"][:, :OW].astype(np.float32)
        out[r0 : r0 + N_FULL_PANELS * PANEL_OUT] = yi
        # Unpack the packed tail: partition 8j+m = tail row m, col block j.
        yti = res.results[i]["yt"].astype(np.float32)
        for j in range(TJ):
            out[
                r0 + N_FULL_PANELS * PANEL_OUT : r0 + ROWS_PER_CORE,
                TSTRIDE * j : TSTRIDE * j + TN,
            ] = yti[TAIL_OUT * j : TAIL_OUT * (j + 1)]
    return out


# revision 31
# speedup vs baseline: 1.2912x; 1.0523x over previous
"""3x3 valid cross-correlation of a 4096x4096 fp32 image + scalar bias,
sharded row-wise across 8 TRN2 NeuronCores.

bf16 datapath (harness gate is rel_err < 2e-2; bf16 lands ~5e-3):
  - x is cast to bf16 on host -> load DMA traffic halves (4.2 MB/core).
  - Matmuls run bf16 x bf16 -> fp32 PSUM at 1 cycle/column.
  - Output is stored as bf16 (4.2 MB/core) and upcast to fp32 on host.

Strategy per core (512 output rows, 514 input rows incl. 2-row halo taken
host-side via overlapping slices -- no device collectives):
  - Row panels of 128 input rows -> 126 output rows (banded matmul):
    out[m, n] = sum_dc sum_dr w[dr, dc] * x[m+dr, n+dc]
    For each kernel column dc, a banded stationary matrix
    B_dc[k, m] = w[k-m, dc] (k-m in 0..2) gives
    psum[m, n] += sum_k B_dc[k, m] * x[k, n+dc].
  - The 8-row tail (rows 504..512) is packed 12 column-blocks deep into
    the PE contraction dim: stationary [120, 96] block-diagonal banded
    matrix, moving operand [120, 345] gathered on host so partition
    10j+i = x[504+i, 341j:341j+345]. 3 matmuls of 343 columns replace a
    full 3x4094-column pass (12x fewer PE cycles for the tail).
  - Scheduling is dominated by the NC activity manager (HAM): DMA is
    capped at ~230-270 GB/s and the PE at ~1.2 GHz until ~5-6us of
    sustained activity earn the full-rate grant (~430 GB/s, 2.4 GHz),
    and any PE idle gap early in the run triggers a half-rate (k=4/8)
    throttle spiral. The warmup matmuls therefore bridge the PE from
    its first possible cycle (~8us, after the fixed SPMD prologue)
    until panel 0's load semaphore fires (~13-14.5us), so the matmul
    stream runs gap-free at full clock (215ns per 512-col matmul).
  - Bias is fused into the PSUM->SBUF drain (ScalarE activation bias for
    even chunks, VectorE tensor_scalar_add for odd ones, both converting
    to bf16).
  - Store rows are padded to 4096 cols (single 8KB packet per row) and
    panels alternate between the gpsimd and sync queues: one queue
    sustains only ~8 concurrent write streams (~200 GB/s).
  - Last core overlaps core 6 by 2 rows so all cores run an identical
    514-row program (4094 = 8*512 - 2).
"""

import numpy as np
import ml_dtypes

import concourse.bacc as bacc
import concourse.mybir as mybir
from concourse import tile
from concourse.bass_utils import run_bass_kernel_spmd

H, W = 4096, 4096
KH, KW = 3, 3
OH, OW = H - KH + 1, W - KW + 1  # 4094, 4094
NCORES = 8
ROWS_PER_CORE = 512              # output rows computed per core
IN_ROWS = ROWS_PER_CORE + KH - 1  # 514 input rows per core
PANEL_OUT = 126                  # output rows per full 128-input-row panel
N_FULL_PANELS = 4                # 4 * 126 = 504
TAIL_OUT = ROWS_PER_CORE - N_FULL_PANELS * PANEL_OUT  # 8
TAIL_IN = TAIL_OUT + KH - 1      # 10
COLS_PER_MM = 512                # PSUM-bank max (512 fp32)
CHUNK = 1024                     # PSUM chunk = 2 banks
# Packed tail geometry: 12 column blocks, stride 341, 343 output columns
# each; 341*11 + 343 = 4094 exactly, and input reads stop at 4096.
TJ = 12
TSTRIDE = 341
TN = 343
WARMUP_MM = 15

_F32 = mybir.dt.float32
_BF16 = mybir.dt.bfloat16
BF = ml_dtypes.bfloat16

_PROGRAM_CACHE = None
last_results = None  # BassKernelResults of the most recent kernel() call


def _build_program():
    nc = bacc.Bacc(
        "TRN2", target_bir_lowering=False, debug=False, num_devices=NCORES
    )
    x = nc.dram_tensor("x", [IN_ROWS, W], _BF16, kind="ExternalInput")
    xt_p = nc.dram_tensor("xt", [TJ * TAIL_IN, TN + KW - 1], _BF16,
                          kind="ExternalInput")
    w = nc.dram_tensor("w", [128, KW * PANEL_OUT], _BF16, kind="ExternalInput")
    wt_p = nc.dram_tensor("wt", [TJ * TAIL_IN, KW * TJ * TAIL_OUT], _BF16,
                          kind="ExternalInput")
    b = nc.dram_tensor("b", [128, 1], _F32, kind="ExternalInput")
    # y rows are padded to 4096 cols so each store row is a single 8KB
    # DMA packet (8188-byte rows split into two ~4KB packets, halving the
    # per-stream DMA rate). Host slices off the 2 pad columns.
    y = nc.dram_tensor("y", [N_FULL_PANELS * PANEL_OUT, W], _BF16,
                       kind="ExternalOutput")
    yt = nc.dram_tensor("yt", [TJ * TAIL_OUT, TN], _BF16,
                        kind="ExternalOutput")

    TK = TJ * TAIL_IN   # 120
    TM = TJ * TAIL_OUT  # 96

    with tile.TileContext(nc) as tc:
        with (
            tc.tile_pool(name="const", bufs=1) as cpool,
            tc.tile_pool(name="xp", bufs=5) as xpool,
            tc.tile_pool(name="op", bufs=3) as opool,
            tc.tile_pool(name="pp", bufs=4, space="PSUM") as ppool,
        ):
            # Warmup memset first on gpsimd (its queue only carries the
            # stores, pushed much later), so the PE can start at once.
            wz = cpool.tile([128, COLS_PER_MM], _BF16)
            nc.gpsimd.memset(wz[:], 0.0)

            # All loads ride the sync queue as full-width DMAs: 4096 bf16
            # cols = one 8KB packet per partition row, the shape that
            # sustains the full ~430 GB/s. Panel 0 goes first; the small
            # constants follow it so the first matmul's weights are ready
            # well before panel 0 completes.
            xts = []
            for panel in range(N_FULL_PANELS):
                xt = xpool.tile([128, W], _BF16)
                xts.append(xt)
            nc.sync.dma_start(xts[0][:], x[0:128, :])
            wt = cpool.tile([128, KW * PANEL_OUT], _BF16)
            nc.sync.dma_start(wt[:], w[:])
            bt = cpool.tile([128, 1], _F32)
            nc.sync.dma_start(bt[:], b[:])
            wtt = cpool.tile([TK, KW * TM], _BF16)
            nc.sync.dma_start(wtt[:], wt_p[:])
            for panel in range(1, N_FULL_PANELS):
                r0 = PANEL_OUT * panel
                nc.sync.dma_start(xts[panel][:], x[r0 : r0 + 128, :])
            xtt = cpool.tile([TK, TN + KW - 1], _BF16)
            nc.sync.dma_start(xtt[:], xt_p[:])

            # PE warmup on zeroed tiles: keeps the PE busy (DVFS ramping)
            # while panel 0 streams in.
            psw = ppool.tile([128, CHUNK], _F32, tag="ps")
            for _ in range(WARMUP_MM):
                nc.tensor.matmul(
                    psw[:126, :COLS_PER_MM],
                    wz[:, :126],
                    wz[:, :],
                    start=True,
                    stop=True,
                )

            def do_panel(panel):
                r0 = PANEL_OUT * panel
                xt = xts[panel]
                ot = opool.tile([128, W], _BF16)
                # Pad columns 4094:4096 so the full 8KB store row is
                # initialized (values are ignored by the host).
                nc.vector.memset(ot[:PANEL_OUT, OW:W], 0.0)
                for c in range(4):
                    ps = ppool.tile([128, CHUNK], _F32, tag="ps")
                    s0 = c * CHUNK
                    sw = min(CHUNK, OW - s0)  # 1024 / 1022
                    for dc in range(KW):
                        for jj in range(2):
                            c0 = s0 + jj * COLS_PER_MM
                            N = min(COLS_PER_MM, OW - c0)
                            lc0 = jj * COLS_PER_MM
                            nc.tensor.matmul(
                                ps[:PANEL_OUT, lc0 : lc0 + N],
                                wt[:128, dc * PANEL_OUT : dc * PANEL_OUT + PANEL_OUT],
                                xt[:128, c0 + dc : c0 + dc + N],
                                start=(dc == 0),
                                stop=(dc == KW - 1),
                            )
                    # Drain PSUM on alternating engines so neither ScalarE
                    # nor VectorE becomes the bottleneck; bias is fused.
                    if c % 2 == 0:
                        nc.scalar.activation(
                            ot[:PANEL_OUT, s0 : s0 + sw],
                            ps[:PANEL_OUT, :sw],
                            mybir.ActivationFunctionType.Identity,
                            bias=bt[:PANEL_OUT, :],
                        )
                    else:
                        nc.vector.tensor_scalar_add(
                            ot[:PANEL_OUT, s0 : s0 + sw],
                            ps[:PANEL_OUT, :sw],
                            bt[:PANEL_OUT, :],
                        )
                # One full-width 8KB-packet store per panel, alternating
                # queues (a single queue sustains only ~8 write streams).
                eng = nc.gpsimd if panel % 2 == 0 else nc.sync
                eng.dma_start(y[r0 : r0 + PANEL_OUT, :], ot[:PANEL_OUT, :W])

            for panel in range(N_FULL_PANELS):
                do_panel(panel)
            # Packed tail: one 3-matmul group covers all 8 tail rows.
            pst = ppool.tile([128, CHUNK], _F32, tag="ps")
            for dc in range(KW):
                nc.tensor.matmul(
                    pst[:TM, :TN],
                    wtt[:TK, dc * TM : dc * TM + TM],
                    xtt[:TK, dc : dc + TN],
                    start=(dc == 0),
                    stop=(dc == KW - 1),
                )
            ott = opool.tile([TM, TN], _BF16)
            nc.scalar.activation(
                ott[:TM, :TN],
                pst[:TM, :TN],
                mybir.ActivationFunctionType.Identity,
                bias=bt[:TM, :],
            )
            nc.gpsimd.dma_start(yt[:, :], ott[:TM, :TN])

    nc.compile()
    return nc


def _banded_weights(weight: np.ndarray) -> np.ndarray:
    """lhsT for each kernel column dc, laid out as [128, KW*PANEL_OUT].

    wT[k, dc*PANEL_OUT + m] = weight[k - m, dc] for 0 <= k - m < KH.
    """
    wT = np.zeros((128, KW * PANEL_OUT), np.float32)
    m = np.arange(PANEL_OUT)
    for dc in range(KW):
        for d in range(KH):
            wT[m + d, dc * PANEL_OUT + m] = weight[d, dc]
    return wT.astype(BF)


def _tail_weights(weight: np.ndarray) -> np.ndarray:
    """Block-diagonal banded stationary for the packed tail.

    S[10j + m + d, dc*96 + 8j + m] = weight[d, dc].
    """
    TK = TJ * TAIL_IN
    TM = TJ * TAIL_OUT
    S = np.zeros((TK, KW * TM), np.float32)
    m = np.arange(TAIL_OUT)
    for dc in range(KW):
        for j in range(TJ):
            for d in range(KH):
                S[TAIL_IN * j + m + d, dc * TM + TAIL_OUT * j + m] = weight[d, dc]
    return S.astype(BF)


def _install_ntff_hook():
    """Shim antenv.axon_hooks so run_bass_kernel_spmd(trace=True) can find
    the axon NTFF profiling hook (the image's antenv lacks axon_hooks)."""
    import sys
    import types

    try:
        from antenv.axon_hooks import get_axon_ntff_profile_hook  # noqa: F401

        return
    except ImportError:
        pass
    import antenv
    from trn_agent_boot.trn_boot import _ntff_profile_via_ctypes

    hook = _ntff_profile_via_ctypes("/opt/axon/libaxon_pjrt.so")
    mod = types.ModuleType("antenv.axon_hooks")
    mod._hook = hook
    mod.set_axon_ntff_profile_hook = lambda h: setattr(mod, "_hook", h)
    mod.get_axon_ntff_profile_hook = lambda: mod._hook
    sys.modules["antenv.axon_hooks"] = mod
    antenv.axon_hooks = mod


def kernel(x, weight, bias, _trace=False, _trace_cores=None):
    global _PROGRAM_CACHE, last_results
    if _trace:
        _install_ntff_hook()
    x = np.asarray(x, dtype=np.float32)
    weight = np.asarray(weight, dtype=np.float32)
    bias = np.asarray(bias, dtype=np.float32)

    if _PROGRAM_CACHE is None:
        _PROGRAM_CACHE = _build_program()
    nc = _PROGRAM_CACHE

    xbf = x.astype(BF)
    wT = _banded_weights(weight)
    wtail = _tail_weights(weight)
    bb = np.full((128, 1), bias[0], np.float32)

    in_maps = []
    for i in range(NCORES):
        r0 = i * ROWS_PER_CORE if i < NCORES - 1 else H - IN_ROWS
        xc = xbf[r0 : r0 + IN_ROWS]
        # Packed tail moving operand: partition 10j+i = tail input row i,
        # column block j (stride TSTRIDE, width TN+2).
        tr = xc[N_FULL_PANELS * PANEL_OUT :]  # rows 504..514
        xtp = np.stack(
            [tr[:, TSTRIDE * j : TSTRIDE * j + TN + KW - 1] for j in range(TJ)]
        ).reshape(TJ * TAIL_IN, TN + KW - 1)
        in_maps.append(
            {
                "x": np.ascontiguousarray(xc),
                "xt": np.ascontiguousarray(xtp),
                "w": wT,
                "wt": wtail,
                "b": bb,
            }
        )

    kwargs = {}
    if _trace:
        kwargs["trace"] = True
        kwargs["trace_cores"] = (
            list(range(NCORES)) if _trace_cores is None else _trace_cores
        )
    res = run_bass_kernel_spmd(nc, in_maps, core_ids=list(range(NCORES)), **kwargs)
    last_results = res

    out = np.empty((OH, OW), np.float32)
    for i in range(NCORES):
        r0 = i * ROWS_PER_CORE if i < NCORES - 1 else H - IN_ROWS
        yi = res.results[i]["y"][:, :OW].astype(np.float32)
        out[r0 : r0 + N_FULL_PANELS * PANEL_OUT] = yi
        # Unpack the packed tail: partition 8j+m = tail row m, col block j.
        yti = res.results[i]["yt"].astype(np.float32)
        for j in range(TJ):
            out[
                r0 + N_FULL_PANELS * PANEL_OUT : r0 + ROWS_PER_CORE,
                TSTRIDE * j : TSTRIDE * j + TN,
            ] = yti[TAIL_OUT * j : TAIL_OUT * (j + 1)]
    return out
